# revision 10
# baseline (speedup 1.0000x reference)
"""Trainium2 Bass kernel for nn_LocalFeatureEncoder.

Computes, for B=8 batches on 8 NeuronCores (batch b -> core b):
    g      = concat(shape_code, structure_code, pose_code)      # (B, 128)
    local  = einsum('kfz,bz->bkf', W, g) + bias                 # (B, 24, 64)
    out    = einsum('btk,bkf->btf', lbs_weights, local)         # (B, 32768, 64)

Per-core device program:
  Stage 1: broadcast g across partitions with a rank-1 PE matmul; compute
    local in "column" layout (kf across partitions) with 12 DVE
    tensor_tensor_reduce ops; PE-transpose + tiny DRAM roundtrip to lay
    local+bias out as a block-diagonal [96, 256] matmul rhs.
  Stage 2: lbs (32768,24) loaded as flat [128, 6144] (partition p holds
    t-rows p*256..p*256+255). Per group of 4 t-rows/partition: PE transpose
    [128,96] -> [96,128], copy PSUM->SBUF, one matmul
    [96,128]^T @ [96,256] -> [128,256] (= out rows t=p*256+g*4+j), copy to
    staging, and one 512KB DMA out per 4 groups.
"""

import os
from contextlib import ExitStack

import numpy as np

import concourse.bass as bass
import concourse.bacc as bacc
import concourse.tile as tile
from concourse import mybir
from concourse import bass_utils

B, T, K, Z, F = 8, 32768, 24, 128, 64
P = 128                 # SBUF partitions
TPB = T // P            # 256 t-rows per partition
CHUNK = 4               # t-rows per group
GCOLS = CHUNK * K       # 96 transpose-input columns per group
NF = CHUNK * F          # 256 matmul output columns per group
NGROUPS = TPB // CHUNK  # 64
NBATCH = 4              # groups per output staging batch
NQ = NGROUPS // NBATCH  # 16 output DMAs
NWTILE = (K * F) // P   # 12 W tiles of [128, 128]
IN_CHUNKS = 8           # lbs load split

_built = {}


def _build(use_f32r: bool):
    key = use_f32r
    if key in _built:
        return _built[key]

    f32 = mybir.dt.float32
    f32r = mybir.dt.float32r
    mm_dt = f32r if use_f32r else f32
    nc = bacc.Bacc("TRN2", target_bir_lowering=False, debug=False)

    lbs_d = nc.dram_tensor("lbs", (P, TPB * K), f32, kind="ExternalInput")
    g_d = nc.dram_tensor("g", (1, Z), f32, kind="ExternalInput")
    w_d = nc.dram_tensor("w", (NWTILE, P, Z), f32, kind="ExternalInput")
    biasc_d = nc.dram_tensor("biasc", (P, NWTILE), f32, kind="ExternalInput")
    ident_d = nc.dram_tensor("ident", (P, P), f32, kind="ExternalInput")
    bdzero_d = nc.dram_tensor("bdzero", (GCOLS, NF), mm_dt, kind="ExternalInput")
    out_d = nc.dram_tensor("out", (P, TPB * F), f32, kind="ExternalOutput")

    with tile.TileContext(nc) as tc, ExitStack() as ctx:
        const = ctx.enter_context(tc.tile_pool(name="const", bufs=1))
        big = ctx.enter_context(tc.tile_pool(name="big", bufs=1))
        dram = ctx.enter_context(
            tc.tile_pool(name="dram", bufs=1, space=bass.MemorySpace.DRAM)
        )
        ps1 = ctx.enter_context(
            tc.tile_pool(name="ps1", bufs=1, space=bass.MemorySpace.PSUM)
        )
        psT = ctx.enter_context(
            tc.tile_pool(name="psT", bufs=3, space=bass.MemorySpace.PSUM)
        )
        psO = ctx.enter_context(
            tc.tile_pool(name="psO", bufs=3, space=bass.MemorySpace.PSUM)
        )
        sbT_pool = ctx.enter_context(tc.tile_pool(name="sbT_pool", bufs=4))
        stag_pool = ctx.enter_context(tc.tile_pool(name="stag_pool", bufs=3))

        # ---- constant loads ----
        ident = const.tile([P, P], f32)
        nc.sync.dma_start(ident[:], ident_d.ap())

        lbs_sb = big.tile([P, TPB * K], f32)
        cw = (TPB * K) // IN_CHUNKS
        for c in range(IN_CHUNKS):
            nc.sync.dma_start(
                lbs_sb[:, c * cw:(c + 1) * cw], lbs_d.ap()[:, c * cw:(c + 1) * cw]
            )

        w_sb = const.tile([P, NWTILE * Z], f32)
        # w_d is (n, p, z); SBUF wants [p, (n z)]
        nc.sync.dma_start(
            w_sb[:].rearrange("p (n z) -> p n z", n=NWTILE),
            w_d.ap().rearrange("n p z -> p n z"),
        )

        g_sb = const.tile([1, Z], f32)
        nc.sync.dma_start(g_sb[:], g_d.ap())

        biascol = const.tile([P, NWTILE], f32)
        nc.sync.dma_start(biascol[:], biasc_d.ap())

        # ---- stage 1: local = W @ g + bias, laid out block-diagonally ----
        ones = const.tile([1, P], f32)
        nc.vector.memset(ones[:], 1.0)
        gb_ps = ps1.tile([P, Z], f32, tag="s1")
        nc.tensor.matmul(gb_ps[:], ones[:], g_sb[:], start=True, stop=True)
        g_bc = const.tile([P, Z], f32)
        nc.scalar.copy(g_bc[:], gb_ps[:])

        localcol = const.tile([P, NWTILE], f32)
        prodw = const.tile([P, NWTILE * Z], f32)
        for n in range(NWTILE):
            nc.vector.tensor_mul(
                prodw[:, n * Z:(n + 1) * Z], w_sb[:, n * Z:(n + 1) * Z], g_bc[:]
            )
        nc.vector.reduce_sum(
            localcol[:],
            prodw[:].rearrange("p (n z) -> p n z", n=NWTILE),
            axis=mybir.AxisListType.X,
        )
        # add bias while in column layout
        nc.vector.tensor_add(localcol[:], localcol[:], biascol[:])

        # transpose [128, 12] -> [12, 128] and roundtrip via DRAM so the
        # (k,f) axis lands on partitions
        lT_ps = ps1.tile([NWTILE, P], f32, tag="s1")
        nc.tensor.transpose(lT_ps[:], localcol[:], ident[:])
        lT_sb = const.tile([NWTILE, P], mm_dt)
        nc.vector.tensor_copy(lT_sb[:], lT_ps[:])
        scratch = dram.tile([NWTILE, P], mm_dt)
        nc.sync.dma_start(scratch[:], lT_sb[:])

        bd = const.tile([GCOLS, NF], mm_dt)
        nc.sync.dma_start(bd[:], bdzero_d.ap())
        for j in range(CHUNK):
            nc.sync.dma_start(
                bd[j * K:(j + 1) * K, j * F:(j + 1) * F],
                scratch[:].rearrange("n (h f) -> (n h) f", h=2),
            )

        # ---- stage 2: main loop over 64 groups ----
        for q in range(NQ):
            stag = stag_pool.tile([P, NBATCH * NF], f32)
            for j in range(NBATCH):
                gi = q * NBATCH + j
                tp = psT.tile([GCOLS, P], f32)
                nc.tensor.transpose(
                    tp[:], lbs_sb[:, gi * GCOLS:(gi + 1) * GCOLS], ident[:]
                )
                sbT = sbT_pool.tile([GCOLS, P], mm_dt)
                if gi % 2 == 0:
                    nc.vector.tensor_copy(sbT[:], tp[:])
                else:
                    nc.scalar.copy(sbT[:], tp[:])
                ops = psO.tile([P, NF], f32)
                nc.tensor.matmul(ops[:], sbT[:], bd[:], start=True, stop=True)
                if gi % 2 == 0:
                    nc.scalar.copy(stag[:, j * NF:(j + 1) * NF], ops[:])
                else:
                    nc.vector.tensor_copy(stag[:, j * NF:(j + 1) * NF], ops[:])
            nc.sync.dma_start(
                out_d.ap()[:, q * NBATCH * NF:(q + 1) * NBATCH * NF], stag[:]
            )

    nc.compile()
    _built[key] = nc
    return nc


def make_in_maps(inputs):
    g_full = np.concatenate(
        [inputs["shape_code"], inputs["structure_code"], inputs["pose_code"]],
        axis=-1,
    ).astype(np.float32)  # (8, 128)
    w_r = np.ascontiguousarray(
        inputs["W"].astype(np.float32).reshape(NWTILE, P, Z)
    )
    # bias in "column" layout: biascol[p, n] = bias.flat[n*128 + p]
    biasc = np.ascontiguousarray(
        inputs["bias"].astype(np.float32).reshape(NWTILE, P).T
    )
    ident = np.eye(P, dtype=np.float32)
    lbs = inputs["lbs_weights"].astype(np.float32)
    in_maps = []
    for b in range(B):
        in_maps.append(
            {
                "lbs": np.ascontiguousarray(lbs[b].reshape(P, TPB * K)),
                "g": g_full[b:b + 1],
                "w": w_r,
                "biasc": biasc,
                "ident": ident,
                "bdzero": np.zeros((GCOLS, NF), dtype=np.float32),
            }
        )
    return in_maps


LAST_RESULT = None


def kernel(**inputs) -> np.ndarray:
    global LAST_RESULT
    use_f32r = os.environ.get("LFE_F32R", "1") == "1"
    nc = _build(use_f32r)
    in_maps = make_in_maps(inputs)
    res = bass_utils.run_bass_kernel_spmd(
        nc,
        in_maps,
        core_ids=list(range(B)),
        trace=os.environ.get("LFE_TRACE", "0") == "1",
    )
    LAST_RESULT = res
    out = np.stack(
        [res.results[b]["out"].reshape(T, F) for b in range(B)], axis=0
    )
    return out


if __name__ == "__main__":
    rng = np.random.default_rng(0)
    inputs = {
        "shape_code": rng.standard_normal((B, 64), dtype=np.float32),
        "structure_code": rng.standard_normal((B, 32), dtype=np.float32),
        "pose_code": rng.standard_normal((B, 32), dtype=np.float32),
        "lbs_weights": rng.random((B, T, K), dtype=np.float32),
        "W": rng.standard_normal((K, F, Z), dtype=np.float32),
        "bias": rng.standard_normal((K, F), dtype=np.float32),
    }
    out = kernel(**inputs)
    g = np.concatenate(
        [inputs["shape_code"], inputs["structure_code"], inputs["pose_code"]], -1
    )
    local = np.einsum("kfz,bz->bkf", inputs["W"], g) + inputs["bias"][None]
    ref = np.einsum("btk,bkf->btf", inputs["lbs_weights"], local)
    err = np.abs(out - ref).max() / np.abs(ref).max()
    print("rel err:", err)
